# revision 1
# baseline (speedup 1.0000x reference)
"""Trainium2 Bass kernel for a char-GRU model.

Model: emb = embed[x]; gi = emb @ W_ih + b_ih  (precomputable per token)
       GRU scan over S=512 steps (h = (1-z)*n + z*h), then h_seq @ W_out + b_out.
Shapes: B=128, S=512, E=16, H=256, V=256.

Sharding: data-parallel over batch across 8 cores (16 batch elems/core),
GRU weights replicated.

Device-side plan per core (all compute on device):
  Phase 1: fused token table T' = embed @ W_ih + biases  ([256, 768]),
           one-hot(x) built via PE broadcast + DVE compare,
           gi^T = T'^T @ onehot as bf16 hi/lo split matmuls (exact selection),
           streamed to DRAM scratch in transposed layout [6, 128, S, 16].
  Phase 2: sequential scan; per step: gh^T = W_hh^T-tiles @ h^T on PE
           (output [gates, batch] so gate math runs on 128 partitions),
           sigmoid/tanh on ACT, gate arithmetic on DVE. h^T history kept in
           SBUF [128, S, 2, 16].
  Phase 3: y = h_seq @ W_out + b_out with h_seq tiles stationary,
           written straight to the [B_l, S, V] output layout.
"""
import os
import sys

for _p in ("/opt/trn_rl_repo", "/root/.axon_site/_ro/trn_rl_repo"):
    if os.path.isdir(_p) and _p not in sys.path:
        sys.path.insert(0, _p)

import numpy as np

import concourse.bacc as bacc
import concourse.mybir as mybir
import concourse.tile as tile
from concourse import bass_utils

F32 = mybir.dt.float32
BF16 = mybir.dt.bfloat16
I32 = mybir.dt.int32
AF = mybir.ActivationFunctionType

B, S, E, H, V = 128, 512, 16, 256, 256
NCORES = 8
BL = B // NCORES          # 16 batch elems per core
G3 = 3 * H                # 768
NM = G3 // 128            # 6 gate tiles of 128
TC = 32                   # gi streaming chunk (steps)
NTOK = S * BL             # 8192 tokens per core
NJC = NTOK // 512         # 16 onehot column chunks

_CACHE: dict = {}


def _build(steps=S, do_p1=True, do_p3=True, cache=True):
    key = (steps, do_p1, do_p3)
    if cache and key in _CACHE:
        return _CACHE[key]
    nc = bacc.Bacc("TRN2", target_bir_lowering=False, debug=False)

    xt_d = nc.dram_tensor("xt", [S, BL], I32, kind="ExternalInput")
    aaug_d = nc.dram_tensor("a_aug", [E + 1, V], F32, kind="ExternalInput")
    baug_d = nc.dram_tensor("b_aug", [E + 1, G3], F32, kind="ExternalInput")
    whh_d = nc.dram_tensor("w_hh", [H, G3], F32, kind="ExternalInput")
    bn_d = nc.dram_tensor("bn", [128, 2 * BL], F32, kind="ExternalInput")
    wout_d = nc.dram_tensor("w_out", [H, V], F32, kind="ExternalInput")
    bout_d = nc.dram_tensor("b_out", [1, V], F32, kind="ExternalInput")
    y_d = nc.dram_tensor("y", [BL, S, V], F32, kind="ExternalOutput")
    gi_d = nc.dram_tensor("gi_scr", [NM, 128, S, BL], F32, kind="Internal")

    with tile.TileContext(nc) as tc:
        with tc.tile_pool(name="consts", bufs=1) as cp:
            whh_sb = cp.tile([128, 2, G3], F32)
            nc.sync.dma_start(whh_sb[:, 0, :], whh_d.ap()[0:128, :])
            nc.sync.dma_start(whh_sb[:, 1, :], whh_d.ap()[128:256, :])
            wout_sb = cp.tile([128, 2, V], F32)
            nc.sync.dma_start(wout_sb[:, 0, :], wout_d.ap()[0:128, :])
            nc.sync.dma_start(wout_sb[:, 1, :], wout_d.ap()[128:256, :])
            bn_sb = cp.tile([128, 2, BL], F32)
            nc.sync.dma_start(bn_sb[:], bn_d.ap().rearrange("p (c b) -> p c b", c=2))
            bout_sb = cp.tile([1, V], F32)
            nc.sync.dma_start(bout_sb[:], bout_d.ap()[:])
            ones = cp.tile([1, 128], F32)
            nc.vector.memset(ones[:], 1.0)
            io_f = cp.tile([128, 2], F32)
            io_i = cp.tile([128, 1], I32)
            nc.gpsimd.iota(io_i[:], pattern=[[0, 1]], base=0, channel_multiplier=1)
            nc.vector.tensor_copy(io_f[:, 0:1], io_i[:])
            nc.vector.tensor_scalar_add(io_f[:, 1:2], io_f[:, 0:1], 128.0)
            # h^T history: [p, s_block(64), chunk(2), s_in_block(8), b]
            # so phase-3 lhsT slices [p, jb, c, :, :] are contiguous 128-col tiles
            hseq = cp.tile([128, S // 8, 2, 8, BL], F32)   # 64KB/partition
            h0 = cp.tile([128, 2, BL], F32)
            nc.vector.memset(h0[:], 0.0)
            # bf16 hi/lo fused token table, persists through phase 1
            thi = cp.tile([128, 2, G3], BF16)
            tlo = cp.tile([128, 2, G3], BF16)

            # ---------------- Phase 1a: token table T' ----------------
            if do_p1:
              with (
                tc.tile_pool(name="p1a", bufs=1) as p1,
                tc.tile_pool(name="ps1a", bufs=2, space="PSUM") as ps1,
            ):
                aaug_sb = p1.tile([E + 1, V], F32)
                nc.sync.dma_start(aaug_sb[:], aaug_d.ap()[:])
                baug_sb = p1.tile([E + 1, G3], F32)
                nc.sync.dma_start(baug_sb[:], baug_d.ap()[:])
                for vc in range(2):
                    for nh in range(2):
                        tp_ps = ps1.tile([128, 384], F32, tag="tp")
                        nc.tensor.matmul(
                            tp_ps[:],
                            aaug_sb[:, vc * 128:(vc + 1) * 128],
                            baug_sb[:, nh * 384:(nh + 1) * 384],
                            start=True, stop=True,
                        )
                        sl = slice(nh * 384, (nh + 1) * 384)
                        nc.vector.tensor_copy(thi[:, vc, sl], tp_ps[:])
                        nc.vector.tensor_sub(tlo[:, vc, sl], tp_ps[:], thi[:, vc, sl])

              # ---------------- Phase 1b/1c: onehot + gi ----------------
              with (
                tc.tile_pool(name="p1b", bufs=1) as pb,
                tc.tile_pool(name="p1st", bufs=3) as pst,
                tc.tile_pool(name="ps1b", bufs=2, space="PSUM") as psb,
            ):
                xi = pb.tile([1, NTOK], I32)
                nc.gpsimd.dma_start(xi[:], xt_d.ap().rearrange("(o s) b -> o (s b)", o=1))
                xf = pb.tile([1, NTOK], F32)
                nc.vector.tensor_copy(xf[:], xi[:])
                oh = pb.tile([128, 2, NTOK], BF16)   # 32KB/partition
                for jc in range(NJC):
                    sl = slice(jc * 512, (jc + 1) * 512)
                    xb_ps = psb.tile([128, 512], F32, tag="xb")
                    nc.tensor.matmul(xb_ps[:], ones[0:1, :], xf[0:1, sl],
                                     start=True, stop=True)
                    for c in range(2):
                        nc.vector.tensor_scalar(
                            oh[:, c, sl], xb_ps[:], io_f[:, c:c + 1], None,
                            op0=mybir.AluOpType.is_equal,
                        )
                # gi^T = T'^T @ onehot, bf16 hi/lo accumulated in fp32 PSUM
                for m in range(6):
                    msl = slice(m * 128, (m + 1) * 128)
                    for jc in range(NJC):
                        jsl = slice(jc * 512, (jc + 1) * 512)
                        g_ps = psb.tile([128, 512], F32, tag="gp")
                        first = True
                        for tt in (thi, tlo):
                            for k in range(2):
                                nc.tensor.matmul(
                                    g_ps[:], tt[:, k, msl], oh[:, k, jsl],
                                    start=first, stop=(tt is tlo and k == 1),
                                )
                                first = False
                        gst = pst.tile([128, 512], F32, tag="gst")
                        nc.vector.tensor_copy(gst[:], g_ps[:])
                        nc.sync.dma_start(
                            gi_d.ap()[m, :, jc * TC:(jc + 1) * TC, :],
                            gst[:].rearrange("p (s b) -> p s b", b=BL),
                        )

            # b_out broadcast to all partitions (one-time, via PE)
            bout_bc = cp.tile([128, V], F32)
            with tc.tile_pool(name="psb0", bufs=1, space="PSUM") as psb0:
                bb_ps = psb0.tile([128, V], F32)
                nc.tensor.matmul(bb_ps[:], ones[0:1, :], bout_sb[0:1, :],
                                 start=True, stop=True)
                nc.vector.tensor_copy(bout_bc[:], bb_ps[:])

            # ---------------- Phase 2: GRU scan (+ fused out-proj) ----------
            y_re = y_d.ap().rearrange("b s v -> s b v")
            with (
                tc.tile_pool(name="gi", bufs=2) as gp,
                tc.tile_pool(name="gates", bufs=3) as ga,
                tc.tile_pool(name="yst", bufs=3) as yp,
                tc.tile_pool(name="ps2", bufs=3, space="PSUM") as ps2,
                tc.tile_pool(name="ps3", bufs=2, space="PSUM") as ps3,
            ):
                gi_sb = None
                for t in range(steps):
                    tci = t % TC
                    if tci == 0:
                        ch = t // TC
                        gi_sb = gp.tile([128, NM, TC, BL], F32, tag="gi")
                        for m in range(6):
                            nc.sync.dma_start(
                                gi_sb[:, m, :, :],
                                gi_d.ap()[m, :, ch * TC:(ch + 1) * TC, :],
                            )
                    if t == 0:
                        hprev = h0
                    else:
                        hprev = hseq[:, (t - 1) // 8, :, (t - 1) % 8, :]
                    # r,z matmuls into their own PSUM bank first, so the
                    # sigmoid chain starts while the n matmuls run
                    gh_rz = ps2.tile([128, 4, BL], F32, tag="ghrz")
                    gh_n = ps2.tile([128, 2, BL], F32, tag="ghn")
                    for m in range(4):
                        for k in range(2):
                            nc.tensor.matmul(
                                gh_rz[:, m, :],
                                whh_sb[:, k, m * 128:(m + 1) * 128],
                                hprev[:, k, :],
                                start=(k == 0), stop=(k == 1),
                            )
                    for m in range(4, 6):
                        for k in range(2):
                            nc.tensor.matmul(
                                gh_n[:, m - 4, :],
                                whh_sb[:, k, m * 128:(m + 1) * 128],
                                hprev[:, k, :],
                                start=(k == 0), stop=(k == 1),
                            )
                    arz = ga.tile([128, 4, BL], F32, tag="arz")
                    nc.vector.tensor_add(arz[:], gh_rz[:], gi_sb[:, 0:4, tci, :])
                    rz = ga.tile([128, 4, BL], F32, tag="rz")
                    nc.scalar.activation(rz[:], arz[:], AF.Sigmoid)
                    nh_t = ga.tile([128, 2, BL], F32, tag="nh")
                    nc.vector.tensor_add(nh_t[:], gh_n[:], bn_sb[:])
                    t1 = ga.tile([128, 2, BL], F32, tag="t1")
                    nc.vector.tensor_mul(t1[:], rz[:, 0:2, :], nh_t[:])
                    t2 = ga.tile([128, 2, BL], F32, tag="t2")
                    nc.vector.tensor_add(t2[:], t1[:], gi_sb[:, 4:6, tci, :])
                    n_t = ga.tile([128, 2, BL], F32, tag="n")
                    nc.scalar.activation(n_t[:], t2[:], AF.Tanh)
                    s_t = ga.tile([128, 2, BL], F32, tag="s")
                    nc.vector.tensor_sub(s_t[:], hprev, n_t[:])
                    sz = ga.tile([128, 2, BL], F32, tag="sz")
                    nc.vector.tensor_mul(sz[:], rz[:, 2:4, :], s_t[:])
                    nc.vector.tensor_add(hseq[:, t // 8, :, t % 8, :], n_t[:], sz[:])

                    # fused out-projection for the finished 8-step block:
                    # fills PE idle time while the next step's gates resolve
                    if do_p3 and t % 8 == 7:
                        jb = t // 8
                        y_ps = ps3.tile([128, V], F32, tag="yps")
                        for c in range(2):
                            nc.tensor.matmul(
                                y_ps[:],
                                hseq[:, jb, c, :, :],
                                wout_sb[:, c, :],
                                start=(c == 0), stop=(c == 1),
                            )
                        yst = yp.tile([128, V], F32, tag="yst")
                        nc.vector.tensor_add(yst[:], y_ps[:], bout_bc[:])
                        nc.sync.dma_start(y_re[jb * 8:(jb + 1) * 8], yst[:])

    nc.compile()
    _CACHE["nc"] = nc
    return nc


def kernel(x, embed, W_ih, b_ih, W_hh, b_hh, W_out, b_out):
    x = np.asarray(x, dtype=np.int32)
    embed = np.asarray(embed, dtype=np.float32)
    W_ih = np.asarray(W_ih, dtype=np.float32)
    b_ih = np.asarray(b_ih, dtype=np.float32)
    W_hh = np.asarray(W_hh, dtype=np.float32)
    b_hh = np.asarray(b_hh, dtype=np.float32)
    W_out = np.asarray(W_out, dtype=np.float32)
    b_out = np.asarray(b_out, dtype=np.float32)

    nc = _build()

    # r,z biases folded into the token table; n-part of b_hh applied in-scan
    bias_combo = b_ih.copy()
    bias_combo[: 2 * H] += b_hh[: 2 * H]
    a_aug = np.concatenate([embed.T, np.ones((1, V), np.float32)], axis=0)
    b_aug = np.concatenate([W_ih, bias_combo[None, :]], axis=0)
    bn = np.ascontiguousarray(
        np.broadcast_to(b_hh[2 * H:].reshape(2, 128).T[:, :, None], (128, 2, BL))
    ).reshape(128, 2 * BL)
    shared = {
        "a_aug": np.ascontiguousarray(a_aug),
        "b_aug": np.ascontiguousarray(b_aug),
        "w_hh": W_hh,
        "bn": bn,
        "w_out": W_out,
        "b_out": np.ascontiguousarray(b_out[None, :]),
    }
    in_maps = []
    for c in range(NCORES):
        xt = np.ascontiguousarray(x[c * BL:(c + 1) * BL, :].T)  # [S, BL]
        in_maps.append({"xt": xt, **shared})

    res = bass_utils.run_bass_kernel_spmd(nc, in_maps, core_ids=list(range(NCORES)))
    y = np.concatenate([res.results[c]["y"] for c in range(NCORES)], axis=0)
    return y.astype(np.float32)



# revision 16
# speedup vs baseline: 1.3433x; 1.3433x over previous
"""Trainium2 Bass kernel for a char-GRU model.

Model: emb = embed[x]; gi = emb @ W_ih + b_ih  (precomputable per token)
       GRU scan over S=512 steps (h = (1-z)*n + z*h), then h_seq @ W_out + b_out.
Shapes: B=128, S=512, E=16, H=256, V=256.

Sharding: data-parallel over batch across 8 cores (16 batch elems/core),
GRU weights replicated.

v2 design (all compute on device, per core):
  Phase 1: fused token table T' = [embed;1] @ [W_ih;bias] as bf16,
           one-hot(x) via PE broadcast + DVE compare, gi^T = T'^T @ onehot
           written straight to SBUF as bf16 [128, 6, S, BL] (no DRAM scratch).
  Phase 2: sequential scan; per step the gi_rz slice and the n-gate bias are
           PRELOADED into PSUM banks (ACT/Pool, off the critical chain), the
           12 fp32 W_hh matmuls accumulate on top (start=False), sigmoid reads
           PSUM directly. Chain: mm -> sigmoid -> t1 -> t2 -> tanh -> t3 -> h;
           (1-z) and z*h are computed off-chain in parallel with tanh.
  Phase 3: y = h_seq @ W_out + b_out every 8 steps, f32, emitted one step late
           so it fills PE idle time without delaying the next step's matmuls.
"""
import os
import sys

for _p in ("/opt/trn_rl_repo", "/root/.axon_site/_ro/trn_rl_repo"):
    if os.path.isdir(_p) and _p not in sys.path:
        sys.path.insert(0, _p)

import numpy as np

import concourse.bacc as bacc
import concourse.mybir as mybir
import concourse.tile as tile
from concourse import bass_utils

F32 = mybir.dt.float32
BF16 = mybir.dt.bfloat16
I32 = mybir.dt.int32
AF = mybir.ActivationFunctionType
ALU = mybir.AluOpType

B, S, E, H, V = 128, 512, 16, 256, 256
NCORES = 8
BL = B // NCORES          # 16 batch elems per core
G3 = 3 * H                # 768
NM = G3 // 128            # 6 gate tiles of 128
NTOK = S * BL             # 8192 tokens per core
HTOK = NTOK // 2          # onehot processed in 2 halves to bound SBUF
NJC = HTOK // 512         # 8 onehot column chunks per half

_CACHE: dict = {}


def _build(cache=True):
    if cache and "nc" in _CACHE:
        return _CACHE["nc"]
    nc = bacc.Bacc("TRN2", target_bir_lowering=False, debug=False)

    xt_d = nc.dram_tensor("xt", [S, BL], I32, kind="ExternalInput")
    aaug_d = nc.dram_tensor("a_aug", [E + 1, V], F32, kind="ExternalInput")
    baug_d = nc.dram_tensor("b_aug", [E + 1, G3], F32, kind="ExternalInput")
    whh_d = nc.dram_tensor("w_hh", [H, G3], F32, kind="ExternalInput")
    bn_d = nc.dram_tensor("bn", [128, 2 * BL], F32, kind="ExternalInput")
    wout_d = nc.dram_tensor("w_out", [H, V], F32, kind="ExternalInput")
    bout_d = nc.dram_tensor("b_out", [1, V], F32, kind="ExternalInput")
    y_d = nc.dram_tensor("y", [BL, S, V], F32, kind="ExternalOutput")

    with tile.TileContext(nc) as tc:
        with tc.tile_pool(name="consts", bufs=1) as cp:
            whh_sb = cp.tile([128, 2, G3], F32)
            nc.sync.dma_start(whh_sb[:, 0, :], whh_d.ap()[0:128, :])
            nc.sync.dma_start(whh_sb[:, 1, :], whh_d.ap()[128:256, :])
            wout_sb = cp.tile([128, 2, V], F32)
            nc.sync.dma_start(wout_sb[:, 0, :], wout_d.ap()[0:128, :])
            nc.sync.dma_start(wout_sb[:, 1, :], wout_d.ap()[128:256, :])
            # bf16 copies for the scan/out-proj matmuls (1 cyc/row vs 4)
            whh_bf = cp.tile([128, 2, G3], BF16)
            nc.vector.tensor_copy(whh_bf[:], whh_sb[:])
            wout_bf = cp.tile([128, 2, V], BF16)
            nc.vector.tensor_copy(wout_bf[:], wout_sb[:])
            bn_sb = cp.tile([128, 2, BL], F32)
            nc.sync.dma_start(bn_sb[:], bn_d.ap().rearrange("p (c b) -> p c b", c=2))
            bout_sb = cp.tile([1, V], F32)
            nc.sync.dma_start(bout_sb[:], bout_d.ap()[:])
            ones = cp.tile([1, 128], F32)
            nc.vector.memset(ones[:], 1.0)
            ones_bf = cp.tile([1, 128], BF16)
            nc.vector.memset(ones_bf[:], 1.0)
            io_f = cp.tile([128, 2], F32)
            io_i = cp.tile([128, 1], I32)
            nc.gpsimd.iota(io_i[:], pattern=[[0, 1]], base=0, channel_multiplier=1)
            nc.vector.tensor_copy(io_f[:, 0:1], io_i[:])
            nc.vector.tensor_scalar_add(io_f[:, 1:2], io_f[:, 0:1], 128.0)
            # h history: [p, s_block(64), chunk(2), s_in_block(8), b] so the
            # out-proj lhsT slices [p, jb, c, :, :] are contiguous 128-col tiles
            hseq = cp.tile([128, S // 8, 2, 8, BL], BF16)  # 32KB/partition
            h0 = cp.tile([128, 2, BL], BF16)
            nc.vector.memset(h0[:], 0.0)
            # gate pre-activations for the whole sequence, resident in SBUF
            gi_sb = cp.tile([128, NM, S, BL], BF16)        # 96KB/partition

            # ---------------- Phase 1a: token table T' (bf16) --------------
            with (
                tc.tile_pool(name="p1", bufs=1) as p1,
                tc.tile_pool(name="ps1", bufs=2, space="PSUM") as ps1,
            ):
                thi = p1.tile([128, 2, G3], BF16)
                aaug_sb = p1.tile([E + 1, V], F32)
                nc.sync.dma_start(aaug_sb[:], aaug_d.ap()[:])
                baug_sb = p1.tile([E + 1, G3], F32)
                nc.sync.dma_start(baug_sb[:], baug_d.ap()[:])
                for vc in range(2):
                    for nh in range(2):
                        tp_ps = ps1.tile([128, 384], F32, tag="tp")
                        nc.tensor.matmul(
                            tp_ps[:],
                            aaug_sb[:, vc * 128:(vc + 1) * 128],
                            baug_sb[:, nh * 384:(nh + 1) * 384],
                            start=True, stop=True,
                        )
                        nc.vector.tensor_copy(
                            thi[:, vc, nh * 384:(nh + 1) * 384], tp_ps[:]
                        )

                # ---------------- Phase 1b: onehot + gi -> SBUF ------------
                QTOK = NTOK // 4       # 2048 tokens per quarter
                for q in range(4):
                    with (
                        tc.tile_pool(name=f"p1b{q}", bufs=1) as pb,
                        tc.tile_pool(name=f"p1o{q}", bufs=3) as ohp,
                        tc.tile_pool(name=f"ps1b{q}", bufs=2, space="PSUM") as psb,
                        tc.tile_pool(name=f"ps1g{q}", bufs=3, space="PSUM") as psg,
                    ):
                        xi = pb.tile([1, QTOK], I32)
                        nc.gpsimd.dma_start(
                            xi[:],
                            xt_d.ap()[q * (S // 4):(q + 1) * (S // 4), :]
                            .rearrange("(o s) b -> o (s b)", o=1),
                        )
                        xf = pb.tile([1, QTOK], BF16)
                        nc.vector.tensor_copy(xf[:], xi[:])
                        for jc in range(QTOK // 512):
                            sl = slice(jc * 512, (jc + 1) * 512)
                            xb_ps = psb.tile([128, 512], F32, tag="xb")
                            nc.tensor.matmul(xb_ps[:], ones_bf[0:1, :], xf[0:1, sl],
                                             start=True, stop=True)
                            oh = ohp.tile([128, 2, 512], BF16, tag="oh")
                            for c in range(2):
                                nc.vector.tensor_scalar(
                                    oh[:, c, :], xb_ps[:], io_f[:, c:c + 1], None,
                                    op0=ALU.is_equal,
                                )
                            # gi^T tile chunk = T'^T @ onehot, straight to SBUF
                            s0 = q * (S // 4) + jc * 32
                            for m in range(NM):
                                msl = slice(m * 128, (m + 1) * 128)
                                g_ps = psg.tile([128, 512], F32, tag="gp")
                                for k in range(2):
                                    nc.tensor.matmul(
                                        g_ps[:], thi[:, k, msl], oh[:, k, :],
                                        start=(k == 0), stop=(k == 1),
                                    )
                                dst = gi_sb[:, m, s0:s0 + 32, :]
                                src = g_ps[:].rearrange("p (s b) -> p s b", b=BL)
                                # GPSIMD has no PSUM port: split PSUM->SBUF
                                # evacuation DVE/ACT 2:4 (DVE also runs the
                                # onehot compares, ACT is otherwise idle)
                                if m in (0, 4):
                                    nc.vector.tensor_copy(dst, src)
                                else:
                                    nc.scalar.copy(dst, src)

            # b_out broadcast to all partitions (one-time, via PE)
            bout_bc = cp.tile([128, V], F32)
            with tc.tile_pool(name="psb0", bufs=1, space="PSUM") as psb0:
                bb_ps = psb0.tile([128, V], F32)
                nc.tensor.matmul(bb_ps[:], ones[0:1, :], bout_sb[0:1, :],
                                 start=True, stop=True)
                nc.vector.tensor_copy(bout_bc[:], bb_ps[:])

            # ---------------- Phase 2: GRU scan (+ fused out-proj) ----------
            y_re = y_d.ap().rearrange("b s v -> s b v")
            with (
                tc.tile_pool(name="ga", bufs=3) as ga,
                tc.tile_pool(name="yst", bufs=2) as yp,
                tc.tile_pool(name="psA", bufs=3, space="PSUM") as psA,
                tc.tile_pool(name="psB", bufs=3, space="PSUM") as psB,
                tc.tile_pool(name="ps3", bufs=2, space="PSUM") as ps3,
            ):
                def emit_outproj(jb):
                    y_ps = ps3.tile([128, V], F32, tag="yps")
                    for c in range(2):
                        nc.tensor.matmul(
                            y_ps[:],
                            hseq[:, jb, c, :, :],
                            wout_bf[:, c, :],
                            start=(c == 0), stop=(c == 1),
                        )
                    yst = yp.tile([128, V], F32, tag="yst")
                    nc.vector.tensor_add(yst[:], y_ps[:], bout_bc[:])
                    nc.sync.dma_start(y_re[jb * 8:(jb + 1) * 8], yst[:])

                def emit_preload(t):
                    # bankA via ScalarE (fast PSUM port), bankB via DVE so the
                    # ACT queue holds only sigma/preA/tanh and tanh never waits
                    bankA = psA.tile([128, 4, BL], F32, tag="A")
                    nc.scalar.copy(bankA[:], gi_sb[:, 0:4, t, :])
                    # bankB regions: [0:2] bn + n-gate matmuls, [2:6] sigmoid
                    # output (PSUM->PSUM keeps the ACT engine on its fast port)
                    bankB = psB.tile([128, 6, BL], F32, tag="B")
                    nc.vector.tensor_copy(bankB[:, 0:2, :], bn_sb[:])
                    return bankA, bankB

                def emit_mms(banks, rhs, is_last):
                    # gh += W_hh^T @ rhs accumulated on top of the preloads;
                    # rz gate tiles first (they gate the sigmoid)
                    bankA, bankB = banks
                    for m in range(4):
                        for k in range(2):
                            nc.tensor.matmul(
                                bankA[:, m, :],
                                whh_bf[:, k, m * 128:(m + 1) * 128],
                                rhs[:, k, :],
                                start=False, stop=(is_last and k == 1),
                                skip_group_check=True,
                            )
                    for m in range(4, 6):
                        for k in range(2):
                            nc.tensor.matmul(
                                bankB[:, m - 4, :],
                                whh_bf[:, k, m * 128:(m + 1) * 128],
                                rhs[:, k, :],
                                start=False, stop=(is_last and k == 1),
                                skip_group_check=True,
                            )

                banks = emit_preload(0)
                for t in range(S):
                    hprev = h0 if t == 0 else hseq[:, (t - 1) // 8, :, (t - 1) % 8, :]
                    bankA, bankB = banks
                    emit_mms(banks, hprev, True)
                    # out-proj of a finished block, one step late: fills PE
                    # idle time without delaying this step's matmuls
                    if t >= 9 and (t - 9) % 8 == 0:
                        emit_outproj((t - 9) // 8)

                    nc.scalar.activation(bankB[:, 2:6, :], bankA[:], AF.Sigmoid)
                    # n-gate chain (r = bankB[2:4], z = bankB[4:6])
                    t1 = ga.tile([128, 2, BL], F32, tag="t1")
                    nc.vector.tensor_mul(t1[:], bankB[:, 2:4, :], bankB[:, 0:2, :])
                    t2 = ga.tile([128, 2, BL], F32, tag="t2")
                    nc.vector.tensor_add(t2[:], t1[:], gi_sb[:, 4:6, t, :])
                    n_t = ga.tile([128, 2, BL], F32, tag="n")
                    nc.scalar.activation(n_t[:], t2[:], AF.Tanh)
                    # off-chain helpers: DVE (GPSIMD cannot read PSUM), they
                    # fill the DVE idle gap while tanh runs
                    zh = ga.tile([128, 2, BL], F32, tag="zh")
                    nc.vector.tensor_mul(zh[:], bankB[:, 4:6, :], hprev)
                    omz = ga.tile([128, 2, BL], F32, tag="omz")
                    nc.vector.tensor_scalar(
                        omz[:], bankB[:, 4:6, :], 1.0, -1.0,
                        op0=ALU.subtract, op1=ALU.mult,
                    )
                    t3 = ga.tile([128, 2, BL], F32, tag="t3")
                    nc.vector.tensor_mul(t3[:], omz[:], n_t[:])
                    nc.vector.tensor_add(hseq[:, t // 8, :, t % 8, :],
                                         t3[:], zh[:])
                    if t + 1 < S:
                        banks = emit_preload(t + 1)

                for jb in (62, 63):
                    emit_outproj(jb)

    nc.compile()
    _CACHE["nc"] = nc
    return nc


def kernel(x, embed, W_ih, b_ih, W_hh, b_hh, W_out, b_out):
    x = np.asarray(x, dtype=np.int32)
    embed = np.asarray(embed, dtype=np.float32)
    W_ih = np.asarray(W_ih, dtype=np.float32)
    b_ih = np.asarray(b_ih, dtype=np.float32)
    W_hh = np.asarray(W_hh, dtype=np.float32)
    b_hh = np.asarray(b_hh, dtype=np.float32)
    W_out = np.asarray(W_out, dtype=np.float32)
    b_out = np.asarray(b_out, dtype=np.float32)

    nc = _build()

    # r,z biases folded into the token table; n-part of b_hh applied in-scan
    bias_combo = b_ih.copy()
    bias_combo[: 2 * H] += b_hh[: 2 * H]
    a_aug = np.concatenate([embed.T, np.ones((1, V), np.float32)], axis=0)
    b_aug = np.concatenate([W_ih, bias_combo[None, :]], axis=0)
    bn = np.ascontiguousarray(
        np.broadcast_to(b_hh[2 * H:].reshape(2, 128).T[:, :, None], (128, 2, BL))
    ).reshape(128, 2 * BL)
    shared = {
        "a_aug": np.ascontiguousarray(a_aug),
        "b_aug": np.ascontiguousarray(b_aug),
        "w_hh": W_hh,
        "bn": bn,
        "w_out": W_out,
        "b_out": np.ascontiguousarray(b_out[None, :]),
    }
    in_maps = []
    for c in range(NCORES):
        xt = np.ascontiguousarray(x[c * BL:(c + 1) * BL, :].T)  # [S, BL]
        in_maps.append({"xt": xt, **shared})

    res = bass_utils.run_bass_kernel_spmd(nc, in_maps, core_ids=list(range(NCORES)))
    y = np.concatenate([res.results[c]["y"] for c in range(NCORES)], axis=0)
    return y.astype(np.float32)


# revision 17
# speedup vs baseline: 1.4693x; 1.0938x over previous
"""Trainium2 Bass kernel for a char-GRU model.

Model: emb = embed[x]; gi = emb @ W_ih + b_ih  (precomputable per token)
       GRU scan over S=512 steps (h = (1-z)*n + z*h), then h_seq @ W_out + b_out.
Shapes: B=128, S=512, E=16, H=256, V=256.

Sharding: data-parallel over batch across 8 cores (16 batch elems/core),
GRU weights replicated.

v2 design (all compute on device, per core):
  Phase 1: fused token table T' = [embed;1] @ [W_ih;bias] as bf16,
           one-hot(x) via PE broadcast + DVE compare, gi^T = T'^T @ onehot
           written straight to SBUF as bf16 [128, 6, S, BL] (no DRAM scratch).
  Phase 2: sequential scan; per step the gi_rz slice and the n-gate bias are
           PRELOADED into PSUM banks (ACT/Pool, off the critical chain), the
           12 fp32 W_hh matmuls accumulate on top (start=False), sigmoid reads
           PSUM directly. Chain: mm -> sigmoid -> t1 -> t2 -> tanh -> t3 -> h;
           (1-z) and z*h are computed off-chain in parallel with tanh.
  Phase 3: y = h_seq @ W_out + b_out every 8 steps, f32, emitted one step late
           so it fills PE idle time without delaying the next step's matmuls.
"""
import os
import sys

for _p in ("/opt/trn_rl_repo", "/root/.axon_site/_ro/trn_rl_repo"):
    if os.path.isdir(_p) and _p not in sys.path:
        sys.path.insert(0, _p)

import numpy as np

import concourse.bacc as bacc
import concourse.mybir as mybir
import concourse.tile as tile
from concourse import bass_utils

F32 = mybir.dt.float32
BF16 = mybir.dt.bfloat16
I32 = mybir.dt.int32
AF = mybir.ActivationFunctionType
ALU = mybir.AluOpType

B, S, E, H, V = 128, 512, 16, 256, 256
NCORES = 8
BL = B // NCORES          # 16 batch elems per core
G3 = 3 * H                # 768
NM = G3 // 128            # 6 gate tiles of 128
NTOK = S * BL             # 8192 tokens per core
HTOK = NTOK // 2          # onehot processed in 2 halves to bound SBUF
NJC = HTOK // 512         # 8 onehot column chunks per half

_CACHE: dict = {}


def _build(cache=True):
    if cache and "nc" in _CACHE:
        return _CACHE["nc"]
    nc = bacc.Bacc("TRN2", target_bir_lowering=False, debug=False)

    xt_d = nc.dram_tensor("xt", [S, BL], I32, kind="ExternalInput")
    aaug_d = nc.dram_tensor("a_aug", [E + 1, V], F32, kind="ExternalInput")
    baug_d = nc.dram_tensor("b_aug", [E + 1, G3], F32, kind="ExternalInput")
    whh_d = nc.dram_tensor("w_hh", [H, G3], F32, kind="ExternalInput")
    bn_d = nc.dram_tensor("bn", [128, 2 * BL], F32, kind="ExternalInput")
    wout_d = nc.dram_tensor("w_out", [H, V], F32, kind="ExternalInput")
    bout_d = nc.dram_tensor("b_out", [1, V], F32, kind="ExternalInput")
    y_d = nc.dram_tensor("y", [BL, S, V], F32, kind="ExternalOutput")

    with tile.TileContext(nc) as tc:
        with tc.tile_pool(name="consts", bufs=1) as cp:
            whh_sb = cp.tile([128, 2, G3], F32)
            nc.sync.dma_start(whh_sb[:, 0, :], whh_d.ap()[0:128, :])
            nc.sync.dma_start(whh_sb[:, 1, :], whh_d.ap()[128:256, :])
            wout_sb = cp.tile([128, 2, V], F32)
            nc.sync.dma_start(wout_sb[:, 0, :], wout_d.ap()[0:128, :])
            nc.sync.dma_start(wout_sb[:, 1, :], wout_d.ap()[128:256, :])
            # bf16 copies for the scan/out-proj matmuls (1 cyc/row vs 4)
            whh_bf = cp.tile([128, 2, G3], BF16)
            nc.vector.tensor_copy(whh_bf[:], whh_sb[:])
            wout_bf = cp.tile([128, 2, V], BF16)
            nc.vector.tensor_copy(wout_bf[:], wout_sb[:])
            bn_sb = cp.tile([128, 2, BL], F32)
            nc.sync.dma_start(bn_sb[:], bn_d.ap().rearrange("p (c b) -> p c b", c=2))
            bout_sb = cp.tile([1, V], F32)
            nc.sync.dma_start(bout_sb[:], bout_d.ap()[:])
            ones = cp.tile([1, 128], F32)
            nc.vector.memset(ones[:], 1.0)
            ones_bf = cp.tile([1, 128], BF16)
            nc.vector.memset(ones_bf[:], 1.0)
            io_f = cp.tile([128, 2], F32)
            io_i = cp.tile([128, 1], I32)
            nc.gpsimd.iota(io_i[:], pattern=[[0, 1]], base=0, channel_multiplier=1)
            nc.vector.tensor_copy(io_f[:, 0:1], io_i[:])
            nc.vector.tensor_scalar_add(io_f[:, 1:2], io_f[:, 0:1], 128.0)
            # h history: [p, s_block(64), chunk(2), s_in_block(8), b] so the
            # out-proj lhsT slices [p, jb, c, :, :] are contiguous 128-col tiles
            hseq = cp.tile([128, S // 8, 2, 8, BL], BF16)  # 32KB/partition
            h0 = cp.tile([128, 2, BL], BF16)
            nc.vector.memset(h0[:], 0.0)
            # gate pre-activations for the whole sequence, resident in SBUF
            gi_sb = cp.tile([128, NM, S, BL], BF16)        # 96KB/partition

            # ---------------- Phase 1a: token table T' (bf16) --------------
            with (
                tc.tile_pool(name="p1", bufs=1) as p1,
                tc.tile_pool(name="ps1", bufs=2, space="PSUM") as ps1,
            ):
                thi = p1.tile([128, 2, G3], BF16)
                aaug_sb = p1.tile([E + 1, V], F32)
                nc.sync.dma_start(aaug_sb[:], aaug_d.ap()[:])
                baug_sb = p1.tile([E + 1, G3], F32)
                nc.sync.dma_start(baug_sb[:], baug_d.ap()[:])
                for vc in range(2):
                    for nh in range(2):
                        tp_ps = ps1.tile([128, 384], F32, tag="tp")
                        nc.tensor.matmul(
                            tp_ps[:],
                            aaug_sb[:, vc * 128:(vc + 1) * 128],
                            baug_sb[:, nh * 384:(nh + 1) * 384],
                            start=True, stop=True,
                        )
                        nc.vector.tensor_copy(
                            thi[:, vc, nh * 384:(nh + 1) * 384], tp_ps[:]
                        )

                # ---------------- Phase 1b: onehot + gi -> SBUF ------------
                QTOK = NTOK // 4       # 2048 tokens per quarter
                for q in range(4):
                    with (
                        tc.tile_pool(name=f"p1b{q}", bufs=1) as pb,
                        tc.tile_pool(name=f"p1o{q}", bufs=3) as ohp,
                        tc.tile_pool(name=f"ps1b{q}", bufs=2, space="PSUM") as psb,
                        tc.tile_pool(name=f"ps1g{q}", bufs=3, space="PSUM") as psg,
                    ):
                        xi = pb.tile([1, QTOK], I32)
                        nc.gpsimd.dma_start(
                            xi[:],
                            xt_d.ap()[q * (S // 4):(q + 1) * (S // 4), :]
                            .rearrange("(o s) b -> o (s b)", o=1),
                        )
                        xf = pb.tile([1, QTOK], BF16)
                        nc.vector.tensor_copy(xf[:], xi[:])
                        for jc in range(QTOK // 512):
                            sl = slice(jc * 512, (jc + 1) * 512)
                            xb_ps = psb.tile([128, 512], F32, tag="xb")
                            nc.tensor.matmul(xb_ps[:], ones_bf[0:1, :], xf[0:1, sl],
                                             start=True, stop=True)
                            oh = ohp.tile([128, 2, 512], BF16, tag="oh")
                            for c in range(2):
                                nc.vector.tensor_scalar(
                                    oh[:, c, :], xb_ps[:], io_f[:, c:c + 1], None,
                                    op0=ALU.is_equal,
                                )
                            # gi^T tile chunk = T'^T @ onehot, straight to SBUF
                            s0 = q * (S // 4) + jc * 32
                            for m in range(NM):
                                msl = slice(m * 128, (m + 1) * 128)
                                g_ps = psg.tile([128, 512], F32, tag="gp")
                                for k in range(2):
                                    nc.tensor.matmul(
                                        g_ps[:], thi[:, k, msl], oh[:, k, :],
                                        start=(k == 0), stop=(k == 1),
                                    )
                                dst = gi_sb[:, m, s0:s0 + 32, :]
                                src = g_ps[:].rearrange("p (s b) -> p s b", b=BL)
                                # GPSIMD has no PSUM port: split PSUM->SBUF
                                # evacuation between DVE and ACT only
                                if m % 2 == 0:
                                    nc.vector.tensor_copy(dst, src)
                                else:
                                    nc.scalar.copy(dst, src)

            # b_out broadcast to all partitions (one-time, via PE)
            bout_bc = cp.tile([128, V], F32)
            with tc.tile_pool(name="psb0", bufs=1, space="PSUM") as psb0:
                bb_ps = psb0.tile([128, V], F32)
                nc.tensor.matmul(bb_ps[:], ones[0:1, :], bout_sb[0:1, :],
                                 start=True, stop=True)
                nc.vector.tensor_copy(bout_bc[:], bb_ps[:])

            # ---------------- Phase 2: GRU scan (+ fused out-proj) ----------
            y_re = y_d.ap().rearrange("b s v -> s b v")
            with (
                tc.tile_pool(name="ga", bufs=3) as ga,
                tc.tile_pool(name="yst", bufs=2) as yp,
                tc.tile_pool(name="psA", bufs=3, space="PSUM") as psA,
                tc.tile_pool(name="psB", bufs=3, space="PSUM") as psB,
                tc.tile_pool(name="ps3", bufs=2, space="PSUM") as ps3,
            ):
                def emit_outproj(jb):
                    y_ps = ps3.tile([128, V], F32, tag="yps")
                    for c in range(2):
                        nc.tensor.matmul(
                            y_ps[:],
                            hseq[:, jb, c, :, :],
                            wout_bf[:, c, :],
                            start=(c == 0), stop=(c == 1),
                        )
                    yst = yp.tile([128, V], F32, tag="yst")
                    nc.vector.tensor_add(yst[:], y_ps[:], bout_bc[:])
                    nc.sync.dma_start(y_re[jb * 8:(jb + 1) * 8], yst[:])

                def emit_preload(t):
                    # bankA via ScalarE (fast PSUM port), bankB via DVE so the
                    # ACT queue holds only sigma/preA/tanh and tanh never waits
                    bankA = psA.tile([128, 4, BL], F32, tag="A")
                    nc.scalar.copy(bankA[:], gi_sb[:, 0:4, t, :])
                    bankB = psB.tile([128, 2, BL], F32, tag="B")
                    nc.vector.tensor_copy(bankB[:], bn_sb[:])
                    return bankA, bankB

                def emit_mms(banks, rhs, is_last):
                    # gh += W_hh^T @ rhs accumulated on top of the preloads;
                    # rz gate tiles first (they gate the sigmoid)
                    bankA, bankB = banks
                    for m in range(4):
                        for k in range(2):
                            nc.tensor.matmul(
                                bankA[:, m, :],
                                whh_bf[:, k, m * 128:(m + 1) * 128],
                                rhs[:, k, :],
                                start=False, stop=(is_last and k == 1),
                                skip_group_check=True,
                            )
                    for m in range(4, 6):
                        for k in range(2):
                            nc.tensor.matmul(
                                bankB[:, m - 4, :],
                                whh_bf[:, k, m * 128:(m + 1) * 128],
                                rhs[:, k, :],
                                start=False, stop=(is_last and k == 1),
                                skip_group_check=True,
                            )

                banks = emit_preload(0)
                for t in range(S):
                    hprev = h0 if t == 0 else hseq[:, (t - 1) // 8, :, (t - 1) % 8, :]
                    bankA, bankB = banks
                    emit_mms(banks, hprev, True)
                    # out-proj of a finished block, one step late: fills PE
                    # idle time without delaying this step's matmuls
                    if t >= 9 and (t - 9) % 8 == 0:
                        emit_outproj((t - 9) // 8)

                    rz = ga.tile([128, 4, BL], F32, tag="rz")
                    nc.scalar.activation(rz[:], bankA[:], AF.Sigmoid)
                    # n-gate chain
                    t1 = ga.tile([128, 2, BL], F32, tag="t1")
                    nc.vector.tensor_mul(t1[:], rz[:, 0:2, :], bankB[:])
                    t2 = ga.tile([128, 2, BL], F32, tag="t2")
                    nc.vector.tensor_add(t2[:], t1[:], gi_sb[:, 4:6, t, :])
                    n_t = ga.tile([128, 2, BL], F32, tag="n")
                    nc.scalar.activation(n_t[:], t2[:], AF.Tanh)
                    # off-chain helpers on Pool, overlap with the n-gate chain
                    zh = ga.tile([128, 2, BL], F32, tag="zh")
                    nc.gpsimd.tensor_mul(zh[:], rz[:, 2:4, :], hprev)
                    omz = ga.tile([128, 2, BL], F32, tag="omz")
                    nc.gpsimd.tensor_scalar(
                        omz[:], rz[:, 2:4, :], 1.0, -1.0,
                        op0=ALU.subtract, op1=ALU.mult,
                    )
                    t3 = ga.tile([128, 2, BL], F32, tag="t3")
                    nc.vector.tensor_mul(t3[:], omz[:], n_t[:])
                    nc.vector.tensor_add(hseq[:, t // 8, :, t % 8, :],
                                         t3[:], zh[:])
                    if t + 1 < S:
                        banks = emit_preload(t + 1)

                for jb in (62, 63):
                    emit_outproj(jb)

    nc.compile()
    _CACHE["nc"] = nc
    return nc


def kernel(x, embed, W_ih, b_ih, W_hh, b_hh, W_out, b_out):
    x = np.asarray(x, dtype=np.int32)
    embed = np.asarray(embed, dtype=np.float32)
    W_ih = np.asarray(W_ih, dtype=np.float32)
    b_ih = np.asarray(b_ih, dtype=np.float32)
    W_hh = np.asarray(W_hh, dtype=np.float32)
    b_hh = np.asarray(b_hh, dtype=np.float32)
    W_out = np.asarray(W_out, dtype=np.float32)
    b_out = np.asarray(b_out, dtype=np.float32)

    nc = _build()

    # r,z biases folded into the token table; n-part of b_hh applied in-scan
    bias_combo = b_ih.copy()
    bias_combo[: 2 * H] += b_hh[: 2 * H]
    a_aug = np.concatenate([embed.T, np.ones((1, V), np.float32)], axis=0)
    b_aug = np.concatenate([W_ih, bias_combo[None, :]], axis=0)
    bn = np.ascontiguousarray(
        np.broadcast_to(b_hh[2 * H:].reshape(2, 128).T[:, :, None], (128, 2, BL))
    ).reshape(128, 2 * BL)
    shared = {
        "a_aug": np.ascontiguousarray(a_aug),
        "b_aug": np.ascontiguousarray(b_aug),
        "w_hh": W_hh,
        "bn": bn,
        "w_out": W_out,
        "b_out": np.ascontiguousarray(b_out[None, :]),
    }
    in_maps = []
    for c in range(NCORES):
        xt = np.ascontiguousarray(x[c * BL:(c + 1) * BL, :].T)  # [S, BL]
        in_maps.append({"xt": xt, **shared})

    res = bass_utils.run_bass_kernel_spmd(nc, in_maps, core_ids=list(range(NCORES)))
    y = np.concatenate([res.results[c]["y"] for c in range(NCORES)], axis=0)
    return y.astype(np.float32)
